# revision 2
# baseline (speedup 1.0000x reference)
"""Trainium2 Bass kernel for nn_LossWithBeliveMaps — v6.

loss * N = sum(pred^2) - 2*sum(pred * bm) + sum(bm^2), bm = Ay^T Bx rank-100.

Device: sum(pred^2) (fused square+accumulate per chunk, split ACT/DVE) and
cross = sum(V . Bx) with V = AyT^T @ pred on the PE.  sum(bm^2) is exact on
the host (coordinates only).

DMA plan (v5): hybrid streams.  The SWDGE f32->bf16 cast path has a
straggler taper at stream end (slow SDMA engines drain last), so the first
5 MiB go via SWDGE cast and the final 3 MiB via taper-free HWDGE f32
chunks issued mid-stream from the scalar ring; those blocks get a small
DVE bf16 cast for the matmul while their sum(pred^2) reads the f32
directly.  Contiguous-row partition mapping keeps 1 descriptor/partition.
Factor tables fold (coord - row) into the ACT bias port; all
coordinate-derived tables are host-precomputed f16/f32 constants.

Approximations (validated ~3e-6 total rel err vs 2e-2 gate): unmasked
Gaussian / no dedup in the cross term, bf16 pred and factors.
Sharding: data-parallel over batch, 2 images per core, 8 cores.
"""

import numpy as np

import concourse.bass as bass
import concourse.bacc as bacc
import concourse.mybir as mybir
from concourse import tile
from concourse.bass_utils import run_bass_kernel_spmd

F32 = mybir.dt.float32
F16 = mybir.dt.float16
I32 = mybir.dt.int32
BF16 = mybir.dt.bfloat16
OP = mybir.AluOpType
AF = mybir.ActivationFunctionType

B, H, W = 16, 1024, 1024
NKP = 100
NCORES = 8
IMGS = B // NCORES          # 2 images per core
NRB = 8                     # factor blocks per image (128 rows each)
SC = 0.3535533905932738     # 1/(2*sqrt(2))
CROSS_SCALE = -float(np.pi) / 2.0     # -2 * (pi/4): undo (2/sqrt(pi))^2

# (img, r0, nb, path): chunk covers rows [r0, r0+128*nb); partition p holds
# rows r0 + nb*p + b (contiguous -> 1 DMA descriptor per partition).
# path: 0 = SWDGE f32->bf16 cast; 1 = HWDGE f32 on sync, issued at start
# (pipeline head); 2 = HWDGE f32 on the scalar ring, issued only after
# img0's last chunk arrives (keeps the shared HBM stream need-ordered and
# the SWDGE straggler taper off the critical tail).
CHUNKS = [
    (0, 0, 1, 1), (0, 128, 1, 0), (0, 256, 2, 0), (0, 512, 4, 0),
    (1, 0, 4, 0),
    (1, 512, 2, 2), (1, 768, 1, 2), (1, 896, 1, 2),
]
NG = len(CHUNKS)
NOUT = NG + IMGS            # sumsq cols + 2 cross cols
ACT_SUMSQ = (3, 4)          # these chunks' sum(pred^2) on ACT, rest DVE
NCA = IMGS * NKP + W        # f16 const: y-broadcasts + iota row
NCB = IMGS * NRB + IMGS     # f32 const: block biases + dense-x biases


def _block_list(img):
    out = []
    for g, (im, r0, nb, path) in enumerate(CHUNKS):
        if im != img:
            continue
        for b in range(nb):
            out.append((g, r0, nb, b))
    return out


def build_nc():
    nc = bacc.Bacc(None, target_bir_lowering=False)

    pred = nc.dram_tensor("pred", [IMGS, H, W], F32, kind="ExternalInput")
    csta_d = nc.dram_tensor("csta", [128, NCA], F16, kind="ExternalInput")
    cstb_d = nc.dram_tensor("cstb", [128, NCB], F32, kind="ExternalInput")
    out = nc.dram_tensor("out", [128, NOUT], F32, kind="ExternalOutput")

    with tile.TileContext(nc) as tc:
        with (
            tc.tile_pool(name="const", bufs=1) as constp,
            tc.tile_pool(name="fact", bufs=1) as factp,
            tc.tile_pool(name="pred", bufs=1) as predp,
            tc.tile_pool(name="junk", bufs=2) as junkp,
            tc.tile_pool(name="psum", bufs=1, space="PSUM") as psump,
        ):
            acc = constp.tile([128, NOUT], F32)

            csta = constp.tile([128, NCA], F16)
            nc.sync.dma_start(csta[:], csta_d[:])
            cstb = constp.tile([128, NCB], F32)
            nc.sync.dma_start(cstb[:], cstb_d[:])

            def ycol(img):
                return csta[:, img * NKP:(img + 1) * NKP]

            iotar = csta[0:NKP, IMGS * NKP:IMGS * NKP + W]

            def rbias(img, j):
                return cstb[:, img * NRB + j:img * NRB + j + 1]

            def xbias(img):
                o = IMGS * NRB + img
                return cstb[0:NKP, o:o + 1]

            def chunk_src(g):
                img, r0, nb, path = CHUNKS[g]
                return pred[img, r0:r0 + 128 * nb, :].rearrange(
                    "(p b) w -> p b w", b=nb)

            # pipeline-head HWDGE chunk(s) on sync, right after the consts
            ftiles = {}
            for g, (img, r0, nb, path) in enumerate(CHUNKS):
                if path != 1:
                    continue
                pf = predp.tile([128, nb, W], F32, tag=f"pf{g}",
                                name=f"pf{g}", bufs=1)
                nc.sync.dma_start(pf[:], chunk_src(g))
                ftiles[g] = pf

            # SWDGE cast stream in gpsimd order
            ptiles = {}
            for g, (img, r0, nb, path) in enumerate(CHUNKS):
                if path != 0:
                    continue
                pt = predp.tile([128, nb, W], BF16, tag=f"pt{g}",
                                name=f"pt{g}", bufs=1)
                nc.gpsimd.dma_start(pt[:], chunk_src(g))
                ptiles[g] = pt

            # ---- factors: AyT blocks + dense Bx (ACT, bias-folded) ----
            tfac = {}
            bxd = {}
            for img in range(IMGS):
                t8 = factp.tile([128, NRB, NKP], BF16,
                                tag=f"t{img}", name=f"t{img}")
                for j in range(NRB):
                    nc.scalar.activation(t8[:, j, :], ycol(img),
                                         AF.Derivative_Erf,
                                         scale=SC, bias=rbias(img, j))
                tfac[img] = t8
                bx = factp.tile([NKP, W], BF16, tag=f"bx{img}",
                                name=f"bx{img}")
                nc.scalar.activation(bx[:], iotar, AF.Derivative_Erf,
                                     scale=SC, bias=xbias(img))
                bxd[img] = bx

            V = [psump.tile([NKP, W], F32, tag=f"V{i}", name=f"V{i}")
                 for i in range(IMGS)]

            blkidx = {}
            for img in range(IMGS):
                for j, (g, r0, nb, b) in enumerate(_block_list(img)):
                    blkidx[(g, b)] = j

            # ---- main chunk loop ----
            for g, (img, r0, nb, path) in enumerate(CHUNKS):
                if path == 0:
                    pt = ptiles[g]          # bf16, from SWDGE cast
                    ptsq = pt
                else:
                    pf = ftiles[g]
                    ptsq = pf
                    pt = predp.tile([128, nb, W], BF16, tag=f"ptc{g}",
                                    name=f"ptc{g}", bufs=1)
                    nc.vector.tensor_copy(pt[:], pf[:])

                # V += AyT_blk^T @ pred_blk
                ty = tfac[img]
                nblk = len(_block_list(img))
                for b in range(nb):
                    j = blkidx[(g, b)]
                    first, last = j == 0, j == nblk - 1
                    lhs = ty[:, j, :]
                    for s in range(2):
                        nc.tensor.matmul(
                            V[img][:, s * 512:(s + 1) * 512], lhs,
                            pt[:, b, s * 512:(s + 1) * 512],
                            start=first, stop=last)

                # sum(pred^2) -> acc[:, g]
                if g in ACT_SUMSQ:
                    jq = junkp.tile([128, nb, W], BF16, tag=f"jqa{nb}",
                                    name=f"jqa{g}")
                    nc.scalar.activation(jq[:], ptsq[:], AF.Square,
                                         accum_out=acc[:, g:g + 1])
                else:
                    jq = junkp.tile([128, nb, W], BF16, tag=f"jqv{nb}",
                                    name=f"jqv{g}")
                    nc.vector.scalar_tensor_tensor(
                        jq[:], ptsq[:], 1.0, ptsq[:], OP.mult, OP.mult,
                        accum_out=acc[:, g:g + 1])

                # after img0's big chunk is consumed by ACT, launch the tail
                # HWDGE issues (scalar ring: ordered after SQ c3's data wait)
                if g == 3:
                    for g2, (i2, r2, nb2, p2) in enumerate(CHUNKS):
                        if p2 != 2:
                            continue
                        pf = predp.tile([128, nb2, W], F32, tag=f"pf{g2}",
                                        name=f"pf{g2}", bufs=1)
                        nc.scalar.dma_start(pf[:], chunk_src(g2))
                        ftiles[g2] = pf

                # image complete: fused cross reduce
                if g in (3, NG - 1):
                    jx = junkp.tile([NKP, W], BF16, tag="jx", name=f"jx{g}")
                    nc.vector.scalar_tensor_tensor(
                        jx[:], V[img][:], CROSS_SCALE, bxd[img][:],
                        OP.mult, OP.mult,
                        accum_out=acc[0:NKP, NG + img:NG + img + 1])

            nc.sync.dma_start(out[:], acc[:])

    nc.compile()
    return nc


_NC_CACHE = {}


def _get_nc():
    if "nc" not in _NC_CACHE:
        _NC_CACHE["nc"] = build_nc()
    return _NC_CACHE["nc"]


def _make_consts(crds_core):
    csta = np.zeros((128, NCA), dtype=np.float16)
    cstb = np.zeros((128, NCB), dtype=np.float32)
    p = np.arange(128, dtype=np.float32)
    for img in range(IMGS):
        x = crds_core[img, :, 0].astype(np.float32)
        y = crds_core[img, :, 1].astype(np.float32)
        csta[:, img * NKP:(img + 1) * NKP] = y[None, :].astype(np.float16)
        cstb[0:NKP, IMGS * NRB + img] = -SC * x
        for j, (g, r0, nb, b) in enumerate(_block_list(img)):
            cstb[:, img * NRB + j] = -SC * (r0 + nb * p + b)
    csta[:, IMGS * NKP:] = np.arange(W, dtype=np.float16)[None, :]
    return csta, cstb


def _bm2_exact(crds):
    """sum(bm^2) in f64: 9-tap masked kernel, dedup, edge clipping."""
    u9 = np.exp(-(np.arange(-4.0, 5.0) ** 2) / 8.0)
    total = 0.0
    for i in range(crds.shape[0]):
        pts = np.unique(crds[i], axis=0)
        o = np.zeros((2, len(pts), len(pts)))
        for ax in range(2):
            u = np.zeros((len(pts), H + 8))
            idx = pts[:, 1 - ax, None] + np.arange(9)[None, :]
            np.put_along_axis(u, idx, u9[None, :], axis=1)
            uc = u[:, 4:H + 4]
            o[ax] = uc @ uc.T
        total += (o[0] * o[1]).sum()
    return total


def _run(prediction, coordinates, **kw):
    nc = _get_nc()
    pred = np.ascontiguousarray(np.asarray(prediction), dtype=np.float32)
    crds = np.ascontiguousarray(np.asarray(coordinates), dtype=np.int32)
    assert pred.shape == (B, 1, H, W) and crds.shape == (B, NKP, 2)
    in_maps = []
    for core in range(NCORES):
        sl = slice(core * IMGS, (core + 1) * IMGS)
        csta, cstb = _make_consts(crds[sl])
        in_maps.append({
            "pred": np.ascontiguousarray(pred[sl, 0]),
            "csta": csta,
            "cstb": cstb,
        })
    res = run_bass_kernel_spmd(nc, in_maps, core_ids=list(range(NCORES)), **kw)
    total = _bm2_exact(crds)
    for r in res.results:
        o = r["out"].astype(np.float64)
        total += o[:, 0:NG].sum() + o[0:NKP, NG:NOUT].sum()
    loss = np.asarray(total / (B * H * W), dtype=np.float32)
    return loss, res


def kernel(prediction, coordinates, labels=None, gaussian_kernel=None, **kw):
    loss, _ = _run(prediction, coordinates)
    return loss
